# revision 1
# baseline (speedup 1.0000x reference)
"""Trainium2 Bass kernel for nn_AttLayer (attention pooling).

Reference computation (per sample b):
    uit = tanh(x @ W + b)            # [T, D]
    ait = uit @ u                    # [T]
    a   = exp(ait); a /= (sum(a) + 1e-7)
    out = a @ x                      # [D]

Sharding: data-parallel over batch B=32 across 8 cores (4 samples/core);
W/b/u replicated. No cross-core communication.

Layout: the host pre-transposes x per sample (xT [D, T], partition = d)
and casts it to bf16, so the x@W contraction over d maps onto the PE
array with W chunks stationary — no on-chip transpose. Dataflow per
1024-wide t-chunk (2 chunks per sample, 8 per core):
  PE : uitT[e, t] accumulated over 4 K-chunks (bf16, fp32 PSUM)
  ACT: tanh(+ per-partition bias b[e]) PSUM -> SBUF bf16
  PE : ait[1, t] = u-weighted partition reduction (u as weights)
  ACT: ait PSUM -> SBUF row a_row[1, T]
then per sample: a_row -> DRAM bounce -> 0-stride-DMA broadcast to
[128, T]; ACT exp with per-partition accum (softmax denominator lands
in every partition); DVE affine_mul_reduce pools xT * exp directly
into pooled[128, 4]; reciprocal+scale normalizes; DMA out.

The PE stream is software-pipelined one chunk deep: the ait matmuls of
chunk k are interleaved between the uitT groups of chunk k+1 so they
never stall on the tanh (ACT) latency — measured 380 -> 259 ns/matmul.

Bisected-on-HW notes:
 - native DVE TENSOR_TENSOR_REDUCE crashes TRN2
   (NRT_EXEC_UNIT_UNRECOVERABLE); affine_mul_reduce (custom DVE ucode)
   does the same fused multiply+reduce and works.
 - fp32/fp32r moving operands stream at ~2 cycles/column (4-byte
   fetch); bf16 moving operands ~1 cycle/column — hence bf16 matmuls.
 - 0-stride partition-broadcast DMA is legal only from DRAM, so the
   softmax row bounces through a DRAM scratch tile.
"""

import ml_dtypes
import numpy as np

import concourse.bass as bass  # noqa: F401
import concourse.tile as tile
import concourse.mybir as mybir
from concourse import bacc, bass_utils

f32 = mybir.dt.float32
bf16 = mybir.dt.bfloat16
AF = mybir.ActivationFunctionType
ALU = mybir.AluOpType

B, T, D = 32, 2048, 512
NCORES = 8
SPC = B // NCORES        # samples per core
CH = 1024                # t-chunk width (2 PSUM banks)
NCH = SPC * (T // CH)    # pipelined chunks per core (8)
NDC = D // 128           # K-chunks of the contraction (4)
NEC = D // 128           # e-tiles of uitT (4)
EPS = 1e-7


def build():
    nc = bacc.Bacc("TRN2", target_bir_lowering=False, debug=False)

    xT = nc.dram_tensor("xT", [SPC, D, T], bf16, kind="ExternalInput").ap()
    W = nc.dram_tensor("W", [D, D], bf16, kind="ExternalInput").ap()
    b = nc.dram_tensor("b", [D], f32, kind="ExternalInput").ap()
    u = nc.dram_tensor("u", [D], bf16, kind="ExternalInput").ap()
    # out[s, dt, p] == pooled[b=s, d=dt*128+p]; host reshapes to [SPC, D]
    out = nc.dram_tensor("out", [SPC * NDC, 128], f32, kind="ExternalOutput").ap()

    with tile.TileContext(nc) as tc:
        with (
            tc.tile_pool(name="consts", bufs=1) as cpool,
            tc.tile_pool(name="x", bufs=4) as xpool,
            tc.tile_pool(name="th", bufs=8) as thpool,
            tc.tile_pool(name="a", bufs=2) as apool,
            tc.tile_pool(name="s", bufs=2) as spool,
            tc.tile_pool(name="scr", bufs=1) as scrpool,
            tc.tile_pool(name="po", bufs=2) as popool,
            tc.tile_pool(name="dram", bufs=2, space="DRAM") as dpool,
            tc.tile_pool(name="psU", bufs=2, space="PSUM") as psU,
            tc.tile_pool(name="psA", bufs=2, space="PSUM") as psA,
        ):
            xts_all = {}     # s -> [4 xT tiles]
            th_tiles = {}    # (k, ec) -> bf16 tanh tile [128, CH]
            ait_tiles = {}   # k -> PSUM [1, CH]
            a_bs = {}        # s -> SBUF [128, T] broadcast exp tile
            p8s = {}         # s -> pooled8 [128, 8] partials
            css = {}         # s -> chunksum [128, 2]

            def load_sample(s):
                tiles = []
                for dc in range(NDC):
                    xt = xpool.tile([128, T], bf16, tag=f"x{dc}")
                    nc.sync.dma_start(xt[:], xT[s, dc * 128:(dc + 1) * 128, :])
                    tiles.append(xt)
                xts_all[s] = tiles

            # sample 0's x first: the first matmul group needs it
            load_sample(0)

            # ---- constants (loaded once) ----
            w_sb = cpool.tile([128, NDC * D], bf16)  # [128d, (dc, e)]
            for dc in range(NDC):
                nc.sync.dma_start(w_sb[:, dc * D:(dc + 1) * D],
                                  W[dc * 128:(dc + 1) * 128, :])
            b_sb = cpool.tile([128, NEC], f32)
            nc.sync.dma_start(b_sb[:], b.rearrange("(c p) -> p c", p=128))
            u_sb = cpool.tile([128, NEC], bf16)
            nc.sync.dma_start(u_sb[:], u.rearrange("(c p) -> p c", p=128))

            def emit_ait_pair(k, slot):
                """Two of the 8 u-reduction matmuls for chunk k, slot 0-3."""
                for j in range(2):
                    idx = slot * 2 + j
                    h, ec = idx // NEC, idx % NEC
                    hs = slice(h * 512, (h + 1) * 512)
                    nc.tensor.matmul(
                        ait_tiles[k][:, hs], u_sb[:, ec:ec + 1],
                        th_tiles[(k, ec)][:, hs],
                        start=(ec == 0), stop=(ec == NEC - 1),
                    )

            def emit_chunk_tail(k):
                """ait PSUM -> SBUF, broadcast, exp, pooling for chunk k."""
                s, c = k // 2, k % 2
                arow = apool.tile([1, CH], f32, name="a_row", tag="arow")
                nc.scalar.activation(arow[:], ait_tiles[k][:], AF.Copy)
                for ec in range(NEC):
                    del th_tiles[(k, ec)]
                del ait_tiles[k]
                dscr = dpool.tile([1, CH], f32)
                nc.sync.dma_start(dscr[:], arow[:])
                ab = apool.tile([128, CH], f32, tag="ab")
                nc.sync.dma_start(ab[:].unsqueeze(1),
                                  dscr[:].partition_broadcast(128))
                if c == 0:
                    a_bs[s] = apool.tile([128, T], f32, name="a_b", tag="aexp")
                    css[s] = spool.tile([128, 2], f32, name="cs", tag="cs")
                    p8s[s] = popool.tile([128, 2 * NDC], f32, name="p8",
                                         tag="p8")
                csl = slice(c * CH, (c + 1) * CH)
                nc.scalar.activation(a_bs[s][:, csl], ab[:], AF.Exp,
                                     accum_out=css[s][:, c:c + 1])
                for dt in range(NDC):
                    scr = scrpool.tile([128, CH], f32, tag="scr")
                    nc.vector.affine_mul_reduce(
                        out=scr[:], accum_out=p8s[s][:, dt * 2 + c:dt * 2 + c + 1],
                        in0=xts_all[s][dt][:, csl], in1=a_bs[s][:, csl],
                        scale=1.0, bias=0.0)

            def emit_sample_tail(s):
                S128 = spool.tile([128, 1], f32, tag="S128")
                nc.vector.reduce_sum(S128[:], css[s][:],
                                     axis=mybir.AxisListType.X)
                S128e = spool.tile([128, 1], f32, tag="S128e")
                nc.vector.tensor_scalar_add(S128e[:], S128[:], EPS)
                inv128 = spool.tile([128, 1], f32, tag="inv128")
                nc.vector.reciprocal(inv128[:], S128e[:])
                pooled = popool.tile([128, NDC], f32, tag="pooled")
                nc.vector.reduce_sum(
                    pooled[:],
                    p8s[s][:].rearrange("p (dt h) -> p dt h", dt=NDC),
                    axis=mybir.AxisListType.X)
                pooledn = popool.tile([128, NDC], f32, tag="pooledn")
                nc.vector.tensor_scalar_mul(pooledn[:], pooled[:], inv128[:])
                nc.sync.dma_start(
                    out[s * NDC:(s + 1) * NDC, :].transpose([1, 0]), pooledn[:]
                )

            for k in range(NCH):
                s, c = k // 2, k % 2
                if c == 0 and s > 0:
                    load_sample(s)
                ait_tiles[k] = psA.tile([1, CH], f32, name="ait_ps", tag="ait")
                for ec in range(NEC):
                    ps = psU.tile([128, CH], f32)
                    for h in range(2):
                        toff = c * CH + h * 512
                        for dc in range(NDC):
                            nc.tensor.matmul(
                                ps[:, h * 512:(h + 1) * 512],
                                w_sb[:, dc * D + ec * 128:
                                     dc * D + (ec + 1) * 128],
                                xts_all[s][dc][:, toff:toff + 512],
                                start=(dc == 0), stop=(dc == NDC - 1),
                            )
                    th = thpool.tile([128, CH], bf16)
                    nc.scalar.activation(th[:], ps[:], AF.Tanh,
                                         bias=b_sb[:, ec:ec + 1])
                    th_tiles[(k, ec)] = th
                    if k >= 1:
                        emit_ait_pair(k - 1, ec)
                if k == NCH - 1:
                    # final chunk: no next chunk to hide behind; emit now
                    for slot in range(NEC):
                        emit_ait_pair(k, slot)
                if k >= 1:
                    emit_chunk_tail(k - 1)
                    if (k - 1) % 2 == 1:
                        emit_sample_tail((k - 1) // 2)
            emit_chunk_tail(NCH - 1)
            emit_sample_tail(SPC - 1)
    nc.compile()
    return nc


_NC_CACHE = None


def prepare_in_maps(x, W, b, u):
    assert x.shape == (B, T, D) and W.shape == (D, D)
    x = np.ascontiguousarray(x, dtype=np.float32)
    # host-side pre-transpose + bf16 cast: [B, T, D] -> [B, D, T]
    xt = np.ascontiguousarray(
        np.transpose(x, (0, 2, 1)).astype(ml_dtypes.bfloat16))
    W = np.ascontiguousarray(W, dtype=np.float32).astype(ml_dtypes.bfloat16)
    b = np.ascontiguousarray(b, dtype=np.float32)
    u = np.ascontiguousarray(u, dtype=np.float32).astype(ml_dtypes.bfloat16)
    in_maps = []
    for c in range(NCORES):
        shard = xt[c * SPC:(c + 1) * SPC]
        in_maps.append({"xT": shard, "W": W, "b": b, "u": u})
    return in_maps


def kernel(x: np.ndarray, W: np.ndarray, b: np.ndarray, u: np.ndarray) -> np.ndarray:
    global _NC_CACHE
    in_maps = prepare_in_maps(x, W, b, u)

    if _NC_CACHE is None:
        _NC_CACHE = build()
    nc = _NC_CACHE

    res = bass_utils.run_bass_kernel_spmd(
        nc, in_maps, core_ids=list(range(NCORES))
    )
    outs = [r["out"].reshape(SPC, D) for r in res.results]
    return np.concatenate(outs, axis=0).astype(np.float32)


if __name__ == "__main__":
    rng = np.random.default_rng(0)
    x = rng.standard_normal((B, T, D)).astype(np.float32)
    W = (rng.standard_normal((D, D)) / np.sqrt(D)).astype(np.float32)
    b = np.zeros(D, np.float32)
    u = (rng.standard_normal(D) / np.sqrt(D)).astype(np.float32)
    out = kernel(x=x, W=W, b=b, u=u)
    print("out", out.shape, out.dtype, float(np.abs(out).max()))



# revision 2
# speedup vs baseline: 1.0764x; 1.0764x over previous
"""Trainium2 Bass kernel for nn_AttLayer (attention pooling).

Reference computation (per sample b):
    uit = tanh(x @ W + b)            # [T, D]
    ait = uit @ u                    # [T]
    a   = exp(ait); a /= (sum(a) + 1e-7)
    out = a @ x                      # [D]

Sharding: data-parallel over batch B=32 across 8 cores (4 samples/core);
W/b/u replicated. No cross-core communication.

v2 layout (vs the 129us baseline): the uit matmul runs with the x tile
as the STATIONARY operand (xT [d, t] chunks, 128 t's per tile) and W as
the moving operand, so uit comes out of PSUM in [t-partition, e-free]
layout. That moves the u-dot (ait) off the PE onto the DVE as a
free-axis affine_mul_reduce — PE matmul count drops 320 -> 256, and PE
was the bottleneck engine (78.7us busy at a 216ns/512-col issue rate,
LDWEIGHTS fully hidden).

Downstream per half-sample (1024 t's = 8 t-tiles):
  DVE : ait column per t-tile via affine_mul_reduce(uit x u_rep),
        fp32 accum -> ait_h [128, 8]
  PE  : transpose ait_h -> PSUM [8, 128] (t-ordered row pieces)
  ACT : exp PSUM -> SBUF bf16 [8, 128] + accum_out = per-tile exp sums
        (denominator pieces, summed on HOST - no device normalization)
  DMA : bounce [8, 128] -> DRAM (contiguous, t-ordered) -> 0-stride
        partition-broadcast back as a_b [128, 1024] bf16
  DVE : pooling affine_mul_reduce(xT x a_b) all-bf16 (2x DVE rate),
        fp32 accum -> pooled[s][:, dc*2+h]
Host divides pooled partials by (exp-sum + 1e-7). The tail chain of
half k is software-pipelined into half k+1's matmul/amr stream.

All of x (8MB bf16/core) is DMA'd up front from the gpsimd sequencer
(SP sequencer issues DGE at ~600ns/DMA - spreading issue engines keeps
the head short); first matmul only needs the first 4 chunks (~1MB).

Bisected-on-HW notes (inherited):
 - bf16 moving operands stream at ~1 cycle/column; fp8 fails the 2e-2
   accuracy gate (measured 2.6e-2 on host), so the matmul stays bf16.
 - 0-stride partition-broadcast DMA is legal only from DRAM, so the
   softmax row bounces through a DRAM scratch tile.
 - PE p-state ramps (0.65 -> 2.4 GHz over ~3us of continuous work);
   keeping the matmul stream gapless is worth more than instruction
   count.
"""

import ml_dtypes
import numpy as np

import concourse.bass as bass  # noqa: F401
import concourse.tile as tile
import concourse.mybir as mybir
from concourse import bacc, bass_utils

f32 = mybir.dt.float32
bf16 = mybir.dt.bfloat16
AF = mybir.ActivationFunctionType

B, T, D = 32, 2048, 512
NCORES = 8
SPC = B // NCORES        # samples per core (4)
NH = 2                   # halves per sample (t-chunks of 1024)
HT = T // NH             # 1024 t's per half
NDC = D // 128           # d chunks of the contraction (4)
NTT = HT // 128          # t-tiles per half (8)
NHK = SPC * NH           # halves per core (8)
EPS = 1e-7


def build(use_bias: bool):
    nc = bacc.Bacc("TRN2", target_bir_lowering=False, debug=False)

    xh = nc.dram_tensor("xh", [SPC, NH, NDC, 128, HT], bf16,
                        kind="ExternalInput").ap()
    W = nc.dram_tensor("W", [NDC, 128, D], bf16, kind="ExternalInput").ap()
    u_rep = nc.dram_tensor("u_rep", [128, D], bf16, kind="ExternalInput").ap()
    ident = nc.dram_tensor("ident", [128, 128], f32, kind="ExternalInput").ap()
    if use_bias:
        ones1 = nc.dram_tensor("ones1", [1, 128], bf16,
                               kind="ExternalInput").ap()
        b_row = nc.dram_tensor("b_row", [1, D], bf16,
                               kind="ExternalInput").ap()
    # pooled partials: out[s, p, dc*2+h] = sum_t x[s, dc*128+p, t_h] * e^ait
    out = nc.dram_tensor("out", [SPC, 128, 2 * NDC], f32,
                         kind="ExternalOutput").ap()
    # exp-sum pieces: oden[tt, s*2+h] = sum over t-tile tt of e^ait
    oden = nc.dram_tensor("oden", [NTT, NHK], f32, kind="ExternalOutput").ap()

    with tile.TileContext(nc) as tc:
        with (
            tc.tile_pool(name="consts", bufs=1) as cpool,
            tc.tile_pool(name="x", bufs=1) as xpool,
            tc.tile_pool(name="th", bufs=3) as thpool,
            tc.tile_pool(name="scr", bufs=2) as scrpool,
            tc.tile_pool(name="ait", bufs=2) as apool,
            tc.tile_pool(name="aexp", bufs=2) as aepool,
            tc.tile_pool(name="ab", bufs=2) as abpool,
            tc.tile_pool(name="po", bufs=2) as popool,
            tc.tile_pool(name="den", bufs=1) as dnpool,
            tc.tile_pool(name="dram", bufs=2, space="DRAM") as dpool,
            tc.tile_pool(name="psU", bufs=3, space="PSUM") as psU,
            tc.tile_pool(name="psT", bufs=2, space="PSUM") as psT,
        ):
            # ---- constants (sync queue; x tiles go via gpsimd queue) ----
            w_sb = cpool.tile([128, NDC * D], bf16)  # [128d, (dc, e)]
            for dc in range(NDC):
                nc.sync.dma_start(w_sb[:, dc * D:(dc + 1) * D], W[dc])
            u_sb = cpool.tile([128, D], bf16)
            nc.sync.dma_start(u_sb[:], u_rep[:, :])
            id_sb = cpool.tile([128, 128], f32)
            nc.sync.dma_start(id_sb[:], ident[:, :])
            if use_bias:
                ones_sb = cpool.tile([1, 128], bf16)
                nc.sync.dma_start(ones_sb[:], ones1[:, :])
                brow_sb = cpool.tile([1, D], bf16)
                nc.sync.dma_start(brow_sb[:], b_row[:, :])

            # ---- all x tiles up front (gpsimd sequencer) ----
            xts = {}   # (hk, dc) -> [128, HT] bf16
            for hk in range(NHK):
                s, h = hk // NH, hk % NH
                for dc in range(NDC):
                    xt = xpool.tile([128, HT], bf16, name=f"x{hk}_{dc}",
                                    tag=f"x{hk}_{dc}")
                    nc.gpsimd.dma_start(xt[:], xh[s, h, dc])
                    xts[(hk, dc)] = xt

            den_sb = dnpool.tile([NTT, NHK], f32)
            ait_hs = {}     # hk -> [128, NTT] f32 ait accum columns
            ab_s = {}       # hk -> [128, HT] bf16 broadcast exp weights
            pooled = {}     # s -> [128, 2*NDC] f32

            def emit_tail_head(hk):
                """transpose + exp + bounce + broadcast for half hk."""
                s, h = hk // NH, hk % NH
                pt = psT.tile([NTT, 128], f32, name="pt", tag="pt")
                nc.tensor.transpose(pt[:], ait_hs[hk][:], id_sb[:])
                aexp = aepool.tile([NTT, 128], bf16, name="aexp", tag="aexp")
                nc.scalar.activation(aexp[:], pt[:], AF.Exp,
                                     accum_out=den_sb[:, hk:hk + 1])
                dscr = dpool.tile([1, HT], bf16, name="dscr")
                nc.sync.dma_start(
                    dscr[:].rearrange("a (p c) -> (a p) c", p=NTT), aexp[:])
                ab = abpool.tile([128, HT], bf16, name="a_b", tag="ab")
                nc.sync.dma_start(ab[:].unsqueeze(1),
                                  dscr[:].partition_broadcast(128))
                ab_s[hk] = ab
                del ait_hs[hk]

            def emit_pools(hk):
                """4 pooling affine_mul_reduces for half hk (all-bf16)."""
                s, h = hk // NH, hk % NH
                if h == 0:
                    pooled[s] = popool.tile([128, 2 * NDC], f32,
                                            name=f"pool{s}", tag="pool")
                for dc in range(NDC):
                    scr2 = scrpool.tile([128, HT], bf16, name="scr2",
                                        tag="scr2")
                    nc.vector.affine_mul_reduce(
                        out=scr2[:],
                        accum_out=pooled[s][:, dc * 2 + h:dc * 2 + h + 1],
                        in0=xts[(hk, dc)][:], in1=ab_s[hk][:],
                        scale=1.0, bias=0.0)
                del ab_s[hk]
                if h == 1:
                    nc.gpsimd.dma_start(out[s], pooled[s][:])

            for hk in range(NHK):
                s, h = hk // NH, hk % NH
                ait_hs[hk] = apool.tile([128, NTT], f32, name="ait_h",
                                        tag="ait")
                for m in range(NTT // 2):         # psum pairs: 2 t-tiles each
                    ps = psU.tile([128, 1024], f32, name="ps", tag="ps")
                    for sub in range(2):
                        j = m * 2 + sub
                        tsl = slice(j * 128, (j + 1) * 128)
                        esl = slice(sub * D, (sub + 1) * D)
                        for dc in range(NDC):
                            nc.tensor.matmul(
                                ps[:, esl], xts[(hk, dc)][:, tsl],
                                w_sb[:, dc * D:(dc + 1) * D],
                                start=(dc == 0),
                                stop=(dc == NDC - 1 and not use_bias),
                            )
                        if use_bias:
                            nc.tensor.matmul(ps[:, esl], ones_sb[:],
                                             brow_sb[:], start=False,
                                             stop=True)
                    th = thpool.tile([128, 1024], bf16, name="th", tag="th")
                    nc.scalar.activation(th[:], ps[:], AF.Tanh)
                    for sub in range(2):
                        j = m * 2 + sub
                        scr = scrpool.tile([128, D], bf16, name="scr",
                                           tag="scr")
                        nc.vector.affine_mul_reduce(
                            out=scr[:],
                            accum_out=ait_hs[hk][:, j:j + 1],
                            in0=th[:, sub * D:(sub + 1) * D], in1=u_sb[:],
                            scale=1.0, bias=0.0)
                    if m == 0 and hk >= 1:
                        emit_tail_head(hk - 1)
                    if m == 2 and hk >= 1:
                        emit_pools(hk - 1)
            emit_tail_head(NHK - 1)
            emit_pools(NHK - 1)
            nc.sync.dma_start(oden[:, :], den_sb[:])
    nc.compile()
    return nc


_NC_CACHE = {}


def prepare_in_maps(x, W, b, u):
    assert x.shape == (B, T, D) and W.shape == (D, D)
    x = np.ascontiguousarray(x, dtype=np.float32)
    # [B, T, D] -> [B, D, T] -> [B, dc, 128, h, HT] -> [B, h, dc, 128, HT]
    xt = np.transpose(x, (0, 2, 1)).reshape(B, NDC, 128, NH, HT)
    xt = np.ascontiguousarray(
        np.transpose(xt, (0, 3, 1, 2, 4)).astype(ml_dtypes.bfloat16))
    Wb = np.ascontiguousarray(W, dtype=np.float32).astype(
        ml_dtypes.bfloat16).reshape(NDC, 128, D)
    ub = np.ascontiguousarray(u, dtype=np.float32).astype(ml_dtypes.bfloat16)
    u_rep = np.ascontiguousarray(np.tile(ub[None, :], (128, 1)))
    ident = np.eye(128, dtype=np.float32)
    use_bias = bool(np.any(np.asarray(b) != 0))
    in_maps = []
    for c in range(NCORES):
        m = {"xh": xt[c * SPC:(c + 1) * SPC].reshape(SPC, NH, NDC, 128, HT),
             "W": Wb, "u_rep": u_rep, "ident": ident}
        if use_bias:
            m["ones1"] = np.ones((1, 128), dtype=ml_dtypes.bfloat16)
            m["b_row"] = np.ascontiguousarray(
                np.asarray(b, dtype=np.float32).astype(
                    ml_dtypes.bfloat16)).reshape(1, D)
        in_maps.append(m)
    return in_maps, use_bias


def kernel(x: np.ndarray, W: np.ndarray, b: np.ndarray,
           u: np.ndarray) -> np.ndarray:
    in_maps, use_bias = prepare_in_maps(x, W, b, u)

    if use_bias not in _NC_CACHE:
        _NC_CACHE[use_bias] = build(use_bias)
    nc = _NC_CACHE[use_bias]

    res = bass_utils.run_bass_kernel_spmd(
        nc, in_maps, core_ids=list(range(NCORES))
    )
    outs = []
    for r in res.results:
        pooled = r["out"]                       # [SPC, 128, 2*NDC]
        den = r["oden"]                         # [NTT, NHK]
        num = pooled[:, :, 0::2] + pooled[:, :, 1::2]   # [SPC, 128, NDC]
        num = np.transpose(num, (0, 2, 1)).reshape(SPC, D)
        dsum = den.sum(axis=0)                  # [NHK]
        denom = dsum[0::2] + dsum[1::2] + EPS   # [SPC]
        outs.append(num / denom[:, None])
    return np.concatenate(outs, axis=0).astype(np.float32)


if __name__ == "__main__":
    rng = np.random.default_rng(0)
    x = rng.standard_normal((B, T, D)).astype(np.float32)
    W = (rng.standard_normal((D, D)) / np.sqrt(D)).astype(np.float32)
    b = np.zeros(D, np.float32)
    u = (rng.standard_normal(D) / np.sqrt(D)).astype(np.float32)
    out = kernel(x=x, W=W, b=b, u=u)
    print("out", out.shape, out.dtype, float(np.abs(out).max()))
